# revision 43
# baseline (speedup 1.0000x reference)
"""Trainium2 Bass kernel for nn_AMCValueNet (ragged prefix-attention value net).

Math: the reference's [n-1, n, n] masked-softmax prefix attention collapses to
dense ops.  With S = (q @ k.T)/sqrt(d) and E = exp(S) (scores are O(1), no
max-subtraction needed):

  Lc[i,j]  = sum_{k<j} E[i,k]                (row prefix-scan of E)
  Bt[i,j]  = 1{i<j} / Lc[i,j]
  Ct[k,j]  = sum_i E[i,k] Bt[i,j]            (one [n,n] matmul)
  t1       = sum_{k,j} Ct[k,j] * z[k] * (1/j) * 1{k<j}
  out      = t1 + w2 . sum_i x_i + n*bc
  where z = v @ w1 = x @ (Wv.T @ w1) + bv.w1,  Wc = [w1 | w2].

The z[k]*(1/j)*1{k<j} factor is a bf16 "zmask" built on-chip (iota ->
reciprocal -> scale -> triangular select) while the input DMAs stream, so t1
is just (elementwise mul) + (ones colsum matmul) + (row reduce).

Sharding: query rows i are split into 8 contiguous bands of 40; each core
computes the full K projection (every band needs all keys) plus its band of
Q/S/E/scan/Bt and a partial t1.  The host sums the 8 partial scalars.
"""

import os
import numpy as np
import ml_dtypes

import concourse.bacc as bacc
import concourse.mybir as mybir
from concourse import tile
from concourse.bass_utils import run_bass_kernel_spmd

N = 320
D = 512
NCORES = 8
B = N // NCORES          # 40 query rows per core
PT = 128                 # partition tile
ND = D // PT             # 4 d-chunks
KT_SIZES = [128, 128, 64]  # k tiles covering 320
SCALE = 1.0 / float(np.sqrt(np.float32(D)))

F32 = mybir.dt.float32
BF16 = mybir.dt.bfloat16
BF16_NP = ml_dtypes.bfloat16

LAST_RESULT = None  # BassKernelResults of the most recent run (for test.py)
_CACHED_NC = None


def _ensure_ntff_hook():
    """Install the antenv.axon_hooks NTFF-profile shim if the container's
    antenv stub lacks it (mirrors trn_boot._ntff_profile_via_ctypes)."""
    import contextlib
    import ctypes
    import sys
    import types

    try:
        from antenv.axon_hooks import get_axon_ntff_profile_hook  # noqa: F401
        return
    except ImportError:
        pass
    so_path = "/opt/axon/libaxon_pjrt.so"
    if not os.path.exists(so_path):
        return
    lib = ctypes.CDLL(so_path)
    if not hasattr(lib, "axon_start_nrt_profile"):
        return
    lib.axon_start_nrt_profile.argtypes = [
        ctypes.POINTER(ctypes.c_int64), ctypes.c_size_t]
    lib.axon_start_nrt_profile.restype = ctypes.c_int64
    lib.axon_stop_nrt_profile.argtypes = [ctypes.c_char_p]
    lib.axon_stop_nrt_profile.restype = ctypes.c_int64

    @contextlib.contextmanager
    def _hook(output_dir, device_ids):
        import jax
        jax.devices()
        if device_ids:
            ids = (ctypes.c_int64 * len(device_ids))(*device_ids)
            rc = lib.axon_start_nrt_profile(ids, len(device_ids))
        else:
            rc = lib.axon_start_nrt_profile(None, 0)
        if rc != 0:
            raise RuntimeError(f"axon_start_nrt_profile rc={rc}")
        try:
            yield
        finally:
            n = lib.axon_stop_nrt_profile(str(output_dir).encode())
            print(f"profile: {n} ntff file(s) -> {output_dir}", file=sys.stderr)

    mod = types.ModuleType("antenv.axon_hooks")
    mod.get_axon_ntff_profile_hook = lambda: _hook
    mod.set_axon_ntff_profile_hook = lambda h: None
    import antenv
    antenv.axon_hooks = mod
    sys.modules["antenv.axon_hooks"] = mod


def _build_nc():
    nc = bacc.Bacc("TRN2", target_bir_lowering=False, debug=False)

    xT = nc.dram_tensor("xT", [PT, ND * N], BF16, kind="ExternalInput")
    xTb = nc.dram_tensor("xTb", [PT, ND * B], BF16, kind="ExternalInput")
    wqT = nc.dram_tensor("wqT", [PT, ND * D], BF16, kind="ExternalInput")
    wkT = nc.dram_tensor("wkT", [PT, ND * D], BF16, kind="ExternalInput")
    smalls = nc.dram_tensor("smalls", [PT, 11], F32, kind="ExternalInput")
    rows = nc.dram_tensor("rows", [1, 2 * D], F32, kind="ExternalInput")
    bmask = nc.dram_tensor("bmask", [B, N], BF16, kind="ExternalInput")
    out_d = nc.dram_tensor("out", [1, 1], F32, kind="ExternalOutput")

    with tile.TileContext(nc) as tc:
        with (
            tc.tile_pool(name="w", bufs=1) as wpool,
            tc.tile_pool(name="act", bufs=1) as apool,
            tc.tile_pool(name="small", bufs=1) as spool,
            tc.tile_pool(name="g", bufs=3) as gpool,
            tc.tile_pool(name="pa", bufs=4, space="PSUM") as pa,   # [128,320]
            tc.tile_pool(name="pb", bufs=2, space="PSUM") as pb,   # [128,40]
            tc.tile_pool(name="pc", bufs=1, space="PSUM") as pc,   # [40,320]
            tc.tile_pool(name="pd", bufs=1, space="PSUM") as pd,   # [1,320]
        ):
            wk_sb = wpool.tile([PT, ND, D], BF16, tag="wk")
            wq_sb = wpool.tile([PT, ND, D], BF16, tag="wq")
            xT_sb = wpool.tile([PT, ND, N], BF16, tag="xT")
            xTb_sb = wpool.tile([PT, ND, B], BF16, tag="xTb")
            sm_sb = spool.tile([PT, 11], F32, tag="sm")
            rows_sb = spool.tile([1, 2 * D], F32, tag="rows")
            zm_sb = wpool.tile([PT, 3, N], BF16, tag="zm")
            bm_sb = spool.tile([B, N], BF16, tag="bm")

            # KT-path quarters on sync (KT rounds chase the DMA);
            # Q-path on scalar; small/late on gpsimd.
            for t in range(ND):
                nc.sync.dma_start(wk_sb[:, t, :], wkT[:, t * D:(t + 1) * D])
                nc.sync.dma_start(xT_sb[:, t, :], xT[:, t * N:(t + 1) * N])
            nc.scalar.dma_start(wq_sb[:, 0:2, :], wqT[:, 0:2 * D])
            nc.scalar.dma_start(wq_sb[:, 2:4, :], wqT[:, 2 * D:])
            nc.scalar.dma_start(xTb_sb[:], xTb[:, :])
            nc.gpsimd.dma_start(sm_sb[:], smalls[:, :])
            nc.gpsimd.dma_start(bm_sb[:], bmask[:, :])
            nc.gpsimd.dma_start(rows_sb[:], rows[:, :])

            ones_bf = spool.tile([PT, 1], BF16, tag="ones_bf")
            nc.gpsimd.memset(ones_bf[:], 1.0)

            # zmask[k, j] = z[k] * (1/j) * 1{k<j}, built on-chip:
            # iota j -> f32 -> 1/j (identical rows), scale by z per
            # partition, then triangular select.
            ji_sb = spool.tile([PT, N], mybir.dt.int32, tag="ji")
            nc.gpsimd.iota(ji_sb[:], pattern=[[1, N]], base=0,
                           channel_multiplier=0)
            jf_sb = spool.tile([PT, N], F32, tag="jf")
            nc.vector.tensor_scalar_max(jf_sb[:], ji_sb[:], 1)
            rr_sb = spool.tile([PT, N], F32, tag="rr")
            nc.vector.reciprocal_approx_fast(out=rr_sb[:], in_=jf_sb[:])
            for kt, kn in enumerate(KT_SIZES):
                with nc.allow_low_precision(reason="bf16 zmask, validated"):
                    nc.vector.tensor_scalar_mul(
                        zm_sb[:kn, kt, :], rr_sb[:kn, :],
                        sm_sb[:kn, 8 + kt:9 + kt])
                nc.gpsimd.affine_select(
                    out=zm_sb[:kn, kt, :], in_=zm_sb[:kn, kt, :],
                    compare_op=mybir.AluOpType.is_gt, fill=0.0,
                    base=-(kt * PT), channel_multiplier=-1, pattern=[[1, N]],
                )

            # ---- projections ----
            kproj_sb = apool.tile([PT, ND, N], BF16, tag="kproj")
            qproj_sb = apool.tile([PT, ND, B], BF16, tag="qproj")
            kt_ps = [pa.tile([PT, N], F32, tag="pa", name=f"ktps{q}")
                     for q in range(ND)]
            for dk in range(ND):
                for q in range(ND):
                    nc.tensor.matmul(
                        kt_ps[q][:], wk_sb[:, dk, q * PT:(q + 1) * PT],
                        xT_sb[:, dk, :],
                        start=(dk == 0), stop=(dk == ND - 1),
                    )
            for q in range(ND):
                if q % 2 == 0:
                    nc.scalar.activation(
                        kproj_sb[:, q, :], kt_ps[q][:],
                        mybir.ActivationFunctionType.Identity,
                        bias=sm_sb[:, 4 + q:5 + q],
                    )
                else:
                    nc.vector.tensor_scalar_add(
                        kproj_sb[:, q, :], kt_ps[q][:],
                        sm_sb[:, 4 + q:5 + q])
            for q in range(ND):
                ps = pb.tile([PT, B], F32, tag="pb")
                for dk in range(ND):
                    nc.tensor.matmul(
                        ps[:], wq_sb[:, dk, q * PT:(q + 1) * PT],
                        xTb_sb[:, dk, :],
                        start=(dk == 0), stop=(dk == ND - 1),
                    )
                nc.vector.tensor_scalar_add(
                    qproj_sb[:, q, :], ps[:], sm_sb[:, q:q + 1])
            # ---- S band [B, N]; E = exp(S/sqrt(d)) in bf16 ----
            s_ps = pc.tile([B, N], F32, tag="pc")
            for q in range(ND):
                nc.tensor.matmul(s_ps[:], qproj_sb[:, q, :], kproj_sb[:, q, :],
                                 start=(q == 0), stop=(q == ND - 1))
            e_sb = apool.tile([B, N], BF16, tag="e")
            nc.scalar.activation(e_sb[:], s_ps[:],
                                 mybir.ActivationFunctionType.Exp, scale=SCALE)

            # ---- prefix-scan -> reciprocal -> masked Bt (bf16) ----
            linc_sb = apool.tile([B, N], F32, tag="linc")
            nc.vector.tensor_tensor_scan(
                out=linc_sb[:, 0:N - 1], data0=e_sb[:, 0:N - 1],
                data1=e_sb[:, 0:N - 1], initial=0.0,
                op0=mybir.AluOpType.add, op1=mybir.AluOpType.bypass,
            )
            rec_sb = apool.tile([B, N], F32, tag="rec")
            nc.vector.reciprocal_approx_fast(
                out=rec_sb[:, 0:N - 1], in_=linc_sb[:, 0:N - 1])
            bt_sb = apool.tile([B, N], BF16, tag="bt")
            nc.gpsimd.memset(bt_sb[:, 0:1], 0.0)
            nc.vector.tensor_mul(bt_sb[:, 1:N], rec_sb[:, 0:N - 1],
                                 bm_sb[:, 1:N])

            # ---- Ct = E.T @ Bt ; G = Ct * zmask ; D = colsum(G) ----
            d_ps = pd.tile([1, N], F32, tag="pd")
            for kt, kn in enumerate(KT_SIZES):
                ct_ps = pa.tile([PT, N], F32, tag="pa")
                nc.tensor.matmul(ct_ps[:kn, :],
                                 e_sb[:, kt * PT:kt * PT + kn], bt_sb[:])
                g_sb = gpool.tile([PT, N], BF16, tag="g")
                nc.vector.tensor_mul(g_sb[:kn, :], ct_ps[:kn, :],
                                     zm_sb[:kn, kt, :])
                nc.tensor.matmul(d_ps[:], ones_bf[:kn, :], g_sb[:kn, :],
                                 start=(kt == 0), stop=(kt == 2))

            # t2 = w2 . xsum (off the critical path)
            junk2 = spool.tile([1, D], F32, tag="junk2")
            t2_sb = spool.tile([1, 1], F32, tag="t2")
            nc.vector.tensor_mul(junk2[:], rows_sb[0:1, 0:D],
                                 rows_sb[0:1, D:2 * D])
            nc.vector.reduce_sum(t2_sb[:], junk2[:], axis=mybir.AxisListType.X)

            # ---- out = sum(D) + t2  (rj is folded into zmask) ----
            t1_sb = spool.tile([1, 1], F32, tag="t1")
            nc.vector.reduce_sum(t1_sb[:], d_ps[:], axis=mybir.AxisListType.X)
            out_sb = spool.tile([1, 1], F32, tag="out")
            nc.vector.tensor_add(out_sb[:], t1_sb[:], t2_sb[:])
            nc.sync.dma_start(out_d[:, :], out_sb[:])

    nc.compile()
    return nc


def _get_nc():
    global _CACHED_NC
    if _CACHED_NC is None:
        _CACHED_NC = _build_nc()
    return _CACHED_NC


def _fold(v, nt):
    """[nt*128] -> [128, nt] fold (v[t*128+p] -> out[p, t])."""
    return np.ascontiguousarray(v.reshape(nt, PT).T.astype(np.float32))


def kernel(**inputs):
    global LAST_RESULT
    x = np.asarray(inputs["x"], np.float32)
    Wq = np.asarray(inputs["Wq"], np.float32)
    bq = np.asarray(inputs["bq"], np.float32)
    Wk = np.asarray(inputs["Wk"], np.float32)
    bk = np.asarray(inputs["bk"], np.float32)
    Wv = np.asarray(inputs["Wv"], np.float32)
    bv = np.asarray(inputs["bv"], np.float32)
    Wc = np.asarray(inputs["Wc"], np.float32)
    bc = np.asarray(inputs["bc"], np.float32)

    w1, w2 = Wc[0, :D], Wc[0, D:]
    z = (x @ (Wv.T @ w1) + bv @ w1).astype(np.float32)
    jidx = np.arange(N)[None, :]

    smalls = np.zeros((PT, 11), np.float32)
    smalls[:, 0:4] = _fold(bq, ND)
    smalls[:, 4:8] = _fold(bk, ND)
    zpad = np.zeros(3 * PT, np.float32)
    zpad[:N] = z
    smalls[:, 8:11] = _fold(zpad, 3)

    xs_row = x.sum(axis=0, dtype=np.float64).astype(np.float32)

    def fold2d(a):  # [(t p), X] -> [p, t*X] partition-folded contiguous
        t = a.shape[0] // PT
        return np.ascontiguousarray(
            a.reshape(t, PT, a.shape[1]).transpose(1, 0, 2).reshape(
                PT, t * a.shape[1]))

    common = {
        "xT": fold2d(np.ascontiguousarray(x.T)).astype(BF16_NP),
        "wqT": fold2d(np.ascontiguousarray(Wq.T)).astype(BF16_NP),
        "wkT": fold2d(np.ascontiguousarray(Wk.T)).astype(BF16_NP),
        "smalls": smalls,
    }
    in_maps = []
    for c in range(NCORES):
        i0 = c * B
        iglob = (i0 + np.arange(B))[:, None]
        m = dict(common)
        m["xTb"] = fold2d(
            np.ascontiguousarray(x[i0:i0 + B].T)).astype(BF16_NP)
        m["bmask"] = (iglob < jidx).astype(np.float32).astype(BF16_NP)
        # t2 = w2 . sum_i x_i must be counted exactly once: only core 0
        rowv = np.zeros((1, 2 * D), np.float32)
        rowv[0, :D] = w2
        if c == 0:
            rowv[0, D:] = xs_row
        m["rows"] = rowv
        in_maps.append(m)

    nc = _get_nc()
    trace = bool(int(os.environ.get("KERNEL_TRACE", "0")))
    trace_cores = None
    if trace:
        try:
            _ensure_ntff_hook()
        except Exception as e:
            print(f"ntff hook shim failed ({e!r}); running untraced")
            trace = False
        if int(os.environ.get("KERNEL_TRACE_ALL", "0")):
            trace_cores = list(range(NCORES))
    res = run_bass_kernel_spmd(
        nc, in_maps, core_ids=list(range(NCORES)),
        trace=trace, trace_cores=trace_cores,
    )
    LAST_RESULT = res
    total = np.float64(0.0)
    for c in range(NCORES):
        total += np.float64(res.results[c]["out"][0, 0])
    total += np.float64(N) * np.float64(bc[0])
    return np.array([total], dtype=np.float32)


# revision 44
# speedup vs baseline: 1.0794x; 1.0794x over previous
"""Trainium2 Bass kernel for nn_AMCValueNet (ragged prefix-attention value net).

Math: the reference's [n-1, n, n] masked-softmax prefix attention collapses to
dense ops.  With S = (q @ k.T)/sqrt(d) and E = exp(S) (scores are O(1), no
max-subtraction needed):

  Lc[i,j]  = sum_{k<j} E[i,k]                (row prefix-scan of E)
  Bt[i,j]  = 1{i<j} / Lc[i,j]
  Ct[k,j]  = sum_i E[i,k] Bt[i,j]            (one [n,n] matmul)
  t1       = sum_{k,j} Ct[k,j] * z[k] * (1/j) * 1{k<j}
  out      = t1 + w2 . sum_i x_i + n*bc
  where z = v @ w1 = x @ (Wv.T @ w1) + bv.w1,  Wc = [w1 | w2].

The z[k]*(1/j)*1{k<j} factor is a bf16 "zmask" built on-chip (iota ->
reciprocal -> scale -> triangular select) while the input DMAs stream, so t1
is just (elementwise mul) + (ones colsum matmul) + (row reduce).

Sharding: query rows i are split into 8 contiguous bands of 40; each core
computes the full K projection (every band needs all keys) plus its band of
Q/S/E/scan/Bt and a partial t1.  The host sums the 8 partial scalars.
"""

import os
import numpy as np
import ml_dtypes

import concourse.bacc as bacc
import concourse.mybir as mybir
from concourse import tile
from concourse.bass_utils import run_bass_kernel_spmd

N = 320
D = 512
NCORES = 8
B = N // NCORES          # 40 query rows per core
PT = 128                 # partition tile
ND = D // PT             # 4 d-chunks
KT_SIZES = [128, 128, 64]  # k tiles covering 320
SCALE = 1.0 / float(np.sqrt(np.float32(D)))

F32 = mybir.dt.float32
BF16 = mybir.dt.bfloat16
BF16_NP = ml_dtypes.bfloat16

LAST_RESULT = None  # BassKernelResults of the most recent run (for test.py)
_CACHED_NC = None


def _ensure_ntff_hook():
    """Install the antenv.axon_hooks NTFF-profile shim if the container's
    antenv stub lacks it (mirrors trn_boot._ntff_profile_via_ctypes)."""
    import contextlib
    import ctypes
    import sys
    import types

    try:
        from antenv.axon_hooks import get_axon_ntff_profile_hook  # noqa: F401
        return
    except ImportError:
        pass
    so_path = "/opt/axon/libaxon_pjrt.so"
    if not os.path.exists(so_path):
        return
    lib = ctypes.CDLL(so_path)
    if not hasattr(lib, "axon_start_nrt_profile"):
        return
    lib.axon_start_nrt_profile.argtypes = [
        ctypes.POINTER(ctypes.c_int64), ctypes.c_size_t]
    lib.axon_start_nrt_profile.restype = ctypes.c_int64
    lib.axon_stop_nrt_profile.argtypes = [ctypes.c_char_p]
    lib.axon_stop_nrt_profile.restype = ctypes.c_int64

    @contextlib.contextmanager
    def _hook(output_dir, device_ids):
        import jax
        jax.devices()
        if device_ids:
            ids = (ctypes.c_int64 * len(device_ids))(*device_ids)
            rc = lib.axon_start_nrt_profile(ids, len(device_ids))
        else:
            rc = lib.axon_start_nrt_profile(None, 0)
        if rc != 0:
            raise RuntimeError(f"axon_start_nrt_profile rc={rc}")
        try:
            yield
        finally:
            n = lib.axon_stop_nrt_profile(str(output_dir).encode())
            print(f"profile: {n} ntff file(s) -> {output_dir}", file=sys.stderr)

    mod = types.ModuleType("antenv.axon_hooks")
    mod.get_axon_ntff_profile_hook = lambda: _hook
    mod.set_axon_ntff_profile_hook = lambda h: None
    import antenv
    antenv.axon_hooks = mod
    sys.modules["antenv.axon_hooks"] = mod


def _build_nc():
    nc = bacc.Bacc("TRN2", target_bir_lowering=False, debug=False)

    xT = nc.dram_tensor("xT", [PT, ND * N], BF16, kind="ExternalInput")
    xTb = nc.dram_tensor("xTb", [PT, ND * B], BF16, kind="ExternalInput")
    wqT = nc.dram_tensor("wqT", [PT, ND * D], BF16, kind="ExternalInput")
    wkT = nc.dram_tensor("wkT", [PT, ND * D], BF16, kind="ExternalInput")
    smalls = nc.dram_tensor("smalls", [PT, 11], F32, kind="ExternalInput")
    rows = nc.dram_tensor("rows", [1, 2 * D], F32, kind="ExternalInput")
    bmask = nc.dram_tensor("bmask", [B, N], BF16, kind="ExternalInput")
    out_d = nc.dram_tensor("out", [1, 1], F32, kind="ExternalOutput")

    with tile.TileContext(nc) as tc:
        with (
            tc.tile_pool(name="w", bufs=1) as wpool,
            tc.tile_pool(name="act", bufs=1) as apool,
            tc.tile_pool(name="small", bufs=1) as spool,
            tc.tile_pool(name="g", bufs=3) as gpool,
            tc.tile_pool(name="pa", bufs=4, space="PSUM") as pa,   # [128,320]
            tc.tile_pool(name="pb", bufs=2, space="PSUM") as pb,   # [128,40]
            tc.tile_pool(name="pc", bufs=1, space="PSUM") as pc,   # [40,320]
            tc.tile_pool(name="pd", bufs=1, space="PSUM") as pd,   # [1,320]
        ):
            wk_sb = wpool.tile([PT, ND, D], BF16, tag="wk")
            wq_sb = wpool.tile([PT, ND, D], BF16, tag="wq")
            xT_sb = wpool.tile([PT, ND, N], BF16, tag="xT")
            xTb_sb = wpool.tile([PT, ND, B], BF16, tag="xTb")
            sm_sb = spool.tile([PT, 11], F32, tag="sm")
            rows_sb = spool.tile([1, 2 * D], F32, tag="rows")
            zm_sb = wpool.tile([PT, 3, N], BF16, tag="zm")
            bm_sb = spool.tile([B, N], BF16, tag="bm")

            # KT-path halves on sync; Q-path on scalar; small/late on gpsimd.
            nc.sync.dma_start(wk_sb[:, 0:2, :], wkT[:, 0:2 * D])
            nc.sync.dma_start(xT_sb[:, 0:2, :], xT[:, 0:2 * N])
            nc.sync.dma_start(wk_sb[:, 2:4, :], wkT[:, 2 * D:])
            nc.sync.dma_start(xT_sb[:, 2:4, :], xT[:, 2 * N:])
            nc.scalar.dma_start(wq_sb[:, 0:2, :], wqT[:, 0:2 * D])
            nc.scalar.dma_start(wq_sb[:, 2:4, :], wqT[:, 2 * D:])
            nc.scalar.dma_start(xTb_sb[:], xTb[:, :])
            nc.gpsimd.dma_start(sm_sb[:], smalls[:, :])
            nc.gpsimd.dma_start(bm_sb[:], bmask[:, :])
            nc.gpsimd.dma_start(rows_sb[:], rows[:, :])

            ones_bf = spool.tile([PT, 1], BF16, tag="ones_bf")
            nc.gpsimd.memset(ones_bf[:], 1.0)

            # zmask[k, j] = z[k] * (1/j) * 1{k<j}, built on-chip:
            # iota j -> f32 -> 1/j (identical rows), scale by z per
            # partition, then triangular select.
            ji_sb = spool.tile([PT, N], mybir.dt.int32, tag="ji")
            nc.gpsimd.iota(ji_sb[:], pattern=[[1, N]], base=0,
                           channel_multiplier=0)
            jf_sb = spool.tile([PT, N], F32, tag="jf")
            nc.vector.tensor_scalar_max(jf_sb[:], ji_sb[:], 1)
            rr_sb = spool.tile([PT, N], F32, tag="rr")
            nc.vector.reciprocal_approx_fast(out=rr_sb[:], in_=jf_sb[:])
            for kt, kn in enumerate(KT_SIZES):
                with nc.allow_low_precision(reason="bf16 zmask, validated"):
                    nc.vector.tensor_scalar_mul(
                        zm_sb[:kn, kt, :], rr_sb[:kn, :],
                        sm_sb[:kn, 8 + kt:9 + kt])
                nc.gpsimd.affine_select(
                    out=zm_sb[:kn, kt, :], in_=zm_sb[:kn, kt, :],
                    compare_op=mybir.AluOpType.is_gt, fill=0.0,
                    base=-(kt * PT), channel_multiplier=-1, pattern=[[1, N]],
                )

            # ---- projections ----
            kproj_sb = apool.tile([PT, ND, N], BF16, tag="kproj")
            qproj_sb = apool.tile([PT, ND, B], BF16, tag="qproj")
            kt_ps = [pa.tile([PT, N], F32, tag="pa", name=f"ktps{q}")
                     for q in range(ND)]
            for half in range(2):
                for q in range(ND):
                    for dk in (2 * half, 2 * half + 1):
                        nc.tensor.matmul(
                            kt_ps[q][:], wk_sb[:, dk, q * PT:(q + 1) * PT],
                            xT_sb[:, dk, :],
                            start=(dk == 0), stop=(dk == ND - 1),
                        )
            for q in range(ND):
                if q % 2 == 0:
                    nc.scalar.activation(
                        kproj_sb[:, q, :], kt_ps[q][:],
                        mybir.ActivationFunctionType.Identity,
                        bias=sm_sb[:, 4 + q:5 + q],
                    )
                else:
                    nc.vector.tensor_scalar_add(
                        kproj_sb[:, q, :], kt_ps[q][:],
                        sm_sb[:, 4 + q:5 + q])
            for q in range(ND):
                ps = pb.tile([PT, B], F32, tag="pb")
                for dk in range(ND):
                    nc.tensor.matmul(
                        ps[:], wq_sb[:, dk, q * PT:(q + 1) * PT],
                        xTb_sb[:, dk, :],
                        start=(dk == 0), stop=(dk == ND - 1),
                    )
                nc.vector.tensor_scalar_add(
                    qproj_sb[:, q, :], ps[:], sm_sb[:, q:q + 1])
            # ---- S band [B, N]; E = exp(S/sqrt(d)) in bf16 ----
            s_ps = pc.tile([B, N], F32, tag="pc")
            for q in range(ND):
                nc.tensor.matmul(s_ps[:], qproj_sb[:, q, :], kproj_sb[:, q, :],
                                 start=(q == 0), stop=(q == ND - 1))
            e_sb = apool.tile([B, N], BF16, tag="e")
            nc.scalar.activation(e_sb[:], s_ps[:],
                                 mybir.ActivationFunctionType.Exp, scale=SCALE)

            # ---- prefix-scan -> reciprocal -> masked Bt (bf16) ----
            linc_sb = apool.tile([B, N], F32, tag="linc")
            nc.vector.tensor_tensor_scan(
                out=linc_sb[:, 0:N - 1], data0=e_sb[:, 0:N - 1],
                data1=e_sb[:, 0:N - 1], initial=0.0,
                op0=mybir.AluOpType.add, op1=mybir.AluOpType.bypass,
            )
            rec_sb = apool.tile([B, N], F32, tag="rec")
            nc.vector.reciprocal_approx_fast(
                out=rec_sb[:, 0:N - 1], in_=linc_sb[:, 0:N - 1])
            bt_sb = apool.tile([B, N], BF16, tag="bt")
            nc.gpsimd.memset(bt_sb[:, 0:1], 0.0)
            nc.vector.tensor_mul(bt_sb[:, 1:N], rec_sb[:, 0:N - 1],
                                 bm_sb[:, 1:N])

            # ---- Ct = E.T @ Bt ; G = Ct * zmask ; D = colsum(G) ----
            d_ps = pd.tile([1, N], F32, tag="pd")
            for kt, kn in enumerate(KT_SIZES):
                ct_ps = pa.tile([PT, N], F32, tag="pa")
                nc.tensor.matmul(ct_ps[:kn, :],
                                 e_sb[:, kt * PT:kt * PT + kn], bt_sb[:])
                g_sb = gpool.tile([PT, N], BF16, tag="g")
                nc.vector.tensor_mul(g_sb[:kn, :], ct_ps[:kn, :],
                                     zm_sb[:kn, kt, :])
                nc.tensor.matmul(d_ps[:], ones_bf[:kn, :], g_sb[:kn, :],
                                 start=(kt == 0), stop=(kt == 2))

            # t2 = w2 . xsum (off the critical path)
            junk2 = spool.tile([1, D], F32, tag="junk2")
            t2_sb = spool.tile([1, 1], F32, tag="t2")
            nc.vector.tensor_mul(junk2[:], rows_sb[0:1, 0:D],
                                 rows_sb[0:1, D:2 * D])
            nc.vector.reduce_sum(t2_sb[:], junk2[:], axis=mybir.AxisListType.X)

            # ---- out = sum(D) + t2  (rj is folded into zmask) ----
            t1_sb = spool.tile([1, 1], F32, tag="t1")
            nc.vector.reduce_sum(t1_sb[:], d_ps[:], axis=mybir.AxisListType.X)
            out_sb = spool.tile([1, 1], F32, tag="out")
            nc.vector.tensor_add(out_sb[:], t1_sb[:], t2_sb[:])
            nc.sync.dma_start(out_d[:, :], out_sb[:])

    nc.compile()
    return nc


def _get_nc():
    global _CACHED_NC
    if _CACHED_NC is None:
        _CACHED_NC = _build_nc()
    return _CACHED_NC


def _fold(v, nt):
    """[nt*128] -> [128, nt] fold (v[t*128+p] -> out[p, t])."""
    return np.ascontiguousarray(v.reshape(nt, PT).T.astype(np.float32))


def kernel(**inputs):
    global LAST_RESULT
    x = np.asarray(inputs["x"], np.float32)
    Wq = np.asarray(inputs["Wq"], np.float32)
    bq = np.asarray(inputs["bq"], np.float32)
    Wk = np.asarray(inputs["Wk"], np.float32)
    bk = np.asarray(inputs["bk"], np.float32)
    Wv = np.asarray(inputs["Wv"], np.float32)
    bv = np.asarray(inputs["bv"], np.float32)
    Wc = np.asarray(inputs["Wc"], np.float32)
    bc = np.asarray(inputs["bc"], np.float32)

    w1, w2 = Wc[0, :D], Wc[0, D:]
    z = (x @ (Wv.T @ w1) + bv @ w1).astype(np.float32)
    jidx = np.arange(N)[None, :]

    smalls = np.zeros((PT, 11), np.float32)
    smalls[:, 0:4] = _fold(bq, ND)
    smalls[:, 4:8] = _fold(bk, ND)
    zpad = np.zeros(3 * PT, np.float32)
    zpad[:N] = z
    smalls[:, 8:11] = _fold(zpad, 3)

    xs_row = x.sum(axis=0, dtype=np.float64).astype(np.float32)

    def fold2d(a):  # [(t p), X] -> [p, t*X] partition-folded contiguous
        t = a.shape[0] // PT
        return np.ascontiguousarray(
            a.reshape(t, PT, a.shape[1]).transpose(1, 0, 2).reshape(
                PT, t * a.shape[1]))

    common = {
        "xT": fold2d(np.ascontiguousarray(x.T)).astype(BF16_NP),
        "wqT": fold2d(np.ascontiguousarray(Wq.T)).astype(BF16_NP),
        "wkT": fold2d(np.ascontiguousarray(Wk.T)).astype(BF16_NP),
        "smalls": smalls,
    }
    in_maps = []
    for c in range(NCORES):
        i0 = c * B
        iglob = (i0 + np.arange(B))[:, None]
        m = dict(common)
        m["xTb"] = fold2d(
            np.ascontiguousarray(x[i0:i0 + B].T)).astype(BF16_NP)
        m["bmask"] = (iglob < jidx).astype(np.float32).astype(BF16_NP)
        # t2 = w2 . sum_i x_i must be counted exactly once: only core 0
        rowv = np.zeros((1, 2 * D), np.float32)
        rowv[0, :D] = w2
        if c == 0:
            rowv[0, D:] = xs_row
        m["rows"] = rowv
        in_maps.append(m)

    nc = _get_nc()
    trace = bool(int(os.environ.get("KERNEL_TRACE", "0")))
    trace_cores = None
    if trace:
        try:
            _ensure_ntff_hook()
        except Exception as e:
            print(f"ntff hook shim failed ({e!r}); running untraced")
            trace = False
        if int(os.environ.get("KERNEL_TRACE_ALL", "0")):
            trace_cores = list(range(NCORES))
    res = run_bass_kernel_spmd(
        nc, in_maps, core_ids=list(range(NCORES)),
        trace=trace, trace_cores=trace_cores,
    )
    LAST_RESULT = res
    total = np.float64(0.0)
    for c in range(NCORES):
        total += np.float64(res.results[c]["out"][0, 0])
    total += np.float64(N) * np.float64(bc[0])
    return np.array([total], dtype=np.float32)
